# revision 14
# baseline (speedup 1.0000x reference)
"""Bass/Tile kernel builder for BSplineField3d (tricubic B-spline interpolation).

Algorithm (per NeuronCore, data-parallel over points):
  Phase 1 (build): from phi [128,128,128,3] build Cy4 in fp16:
      Cy4[x0, yc, z, xs, c, k] = sum_m A[k,m] * phi[x0+xs, yc+m, z, c]
    (x0 in [0,124], xs in [0,4)).  The y-dim B-spline is pre-contracted into
    per-cell polynomial coefficients in v; the 4 x-taps of a point are
    DUPLICATED into every record so that one point needs exactly ONE
    contiguous gather: records of 48 fp16 = [xs4][c3][k4] are contiguous
    along z, so the z-window (4 records = 192 fp16 = 384 B) starting at
    (x0=ix, yc=iy, z=iz) holds everything point-specific.
    Built with fp16 PE matmuls against a banded matrix W[y,(k,yc)], with a
    sliding window of stage tiles (each x-slab feeds 4 stages).
  Phase 2 (points): per chunk of 128x128 points:
    - cell indices + fractional coords on DVE
    - P indirect-DMA gathers (one index per partition per instruction,
      the only vector-mode the HW ucode supports), 384 B per descriptor
    - contraction on DVE in fp16 (packed APs -> 2x perf mode):
        poly-eval in v over k (mult + tree-add), weighted x taps,
        weighted z taps (tree-adds, partially in-place)
"""

from contextlib import ExitStack

import sys as _sys
for _p in ("/opt/trn_rl_repo",):
    if _p not in _sys.path:
        _sys.path.append(_p)

import numpy as np

import concourse.bass as bass
import concourse.tile as tile
from concourse import mybir
from concourse._compat import with_exitstack

F32 = mybir.dt.float32
F16 = mybir.dt.float16
I32 = mybir.dt.int32

NX = 128          # grid points per dim
NCELL = 125       # valid cells per dim (ix in [0,124])
NC_ = 3           # components
ZC = NX * NC_     # 384 floats per (x,y) z-row in phi
REC = 48          # [xs4][c3][k4] fp16 per (x0,yc,z) record in Cy4
ROWE = NX * REC   # 6144 fp16 per (x0,yc)
NRECTOT = NCELL * NCELL * NX   # 2,000,000 records
XSTRIDE = NCELL * NX           # 16000: record-index stride for x0

COLS = 1984       # points per partition (128*1984 = 253952 >= 250000)
P = 124           # points per partition per chunk
NCHUNK = COLS // P  # 16

# spacing: dx = 2/(nx-3) = 2/125 -> 1/dx = 62.5; u = (x+1)*62.5
INV_D = 62.5


def bspline_poly_A():
    """A[k][m]: coefficient of v^k in the cubic B-spline weight of tap m."""
    return np.array(
        [
            [1 / 6, 4 / 6, 1 / 6, 0.0],
            [-3 / 6, 0.0, 3 / 6, 0.0],
            [3 / 6, -6 / 6, 3 / 6, 0.0],
            [-1 / 6, 3 / 6, -3 / 6, 1 / 6],
        ],
        dtype=np.float64,
    )


def build_W_const():
    """W[y, k*125+yc] = A[k, y-yc] for 0 <= y-yc <= 3 else 0. Shape [128, 500]."""
    A = bspline_poly_A()
    W = np.zeros((128, 4, 125), np.float32)
    for yc in range(NCELL):
        for m in range(4):
            for k in range(4):
                W[yc + m, k, yc] = A[k, m]
    return W.reshape(128, 500).astype(np.float16)


def _ap(t, offset, dims):
    """Build a raw AP on the same tensor as AP `t` with explicit [step, num] dims."""
    return bass.AP(tensor=t.tensor, offset=t.offset + offset, ap=[list(d) for d in dims])


@with_exitstack
def bspline_kernel(ctx: ExitStack, tc: tile.TileContext, outs, ins):
    """outs = [T_out [128, COLS, 3] f32]; ins = [xs, ys, zs [128, COLS] f32, phi [128,128,384] f32]."""
    nc = tc.nc
    xs, ys, zs, phi = ins
    t_out = outs[0]

    w_np = build_W_const()
    w_dram = nc.inline_tensor(w_np, name="w_const")

    dram = ctx.enter_context(tc.tile_pool(name="cydram", bufs=1, space="DRAM"))
    cy = dram.tile([NRECTOT, REC], F16)

    add = mybir.AluOpType.add
    sub = mybir.AluOpType.subtract
    mult = mybir.AluOpType.mult
    amin = mybir.AluOpType.min

    # phase-2 prep pools opened early so chunk prep can overlap phase 1
    coords = ctx.enter_context(tc.tile_pool(name="p2_coords", bufs=2))
    small = ctx.enter_context(tc.tile_pool(name="p2_small", bufs=2))
    idxp = ctx.enter_context(tc.tile_pool(name="p2_idx", bufs=2))

    # ---------------- Phase 1: build Cy4 ----------------
    with ExitStack() as p1:
        singles = p1.enter_context(tc.tile_pool(name="p1_singles", bufs=1))
        phis = p1.enter_context(tc.tile_pool(name="p1_phi", bufs=4))
        stages = p1.enter_context(tc.tile_pool(name="p1_stage", bufs=8))
        psums = p1.enter_context(tc.psum_pool(name="p1_psum", bufs=2))

        w_sb = singles.tile([128, 500], F16)
        nc.sync.dma_start(out=w_sb[:], in_=w_dram.ap())

        def slot_ap(st, xsl):
            return _ap(st[:], xsl * 12, [[ROWE, NCELL], [REC, NX], [1, 12]])

        stage_by_x0 = {}
        for x2 in range(NX // 2):
            # cast fp32 -> fp16 during DMA (SWDGE); two x-slabs per load
            phi_x = phis.tile([128, 2 * ZC], F16, name="phi_in")
            nc.gpsimd.dma_start(
                out=phi_x[:],
                in_=_ap(phi, 2 * x2 * NX * ZC,
                        [[ZC, 128], [NX * ZC, 2], [1, ZC]]))
            for xh in range(2):
                x = 2 * x2 + xh
                ps = psums.tile([NCELL, 2048], F32)
                for k in range(4):
                    nc.tensor.matmul(
                        ps[:, k * 512:k * 512 + ZC],
                        w_sb[:, k * NCELL:(k + 1) * NCELL],
                        phi_x[:, xh * ZC:(xh + 1) * ZC],
                        start=True,
                        stop=True,
                    )
                targets = [(x - xsl, xsl) for xsl in range(4)
                           if 0 <= x - xsl <= NCELL - 1]
                for x0, xsl in targets:
                    if x0 not in stage_by_x0:
                        stage_by_x0[x0] = stages.tile([128, ROWE], F16, name="stage")
                # first target: direct fused fp32 psum -> fp16 stage copy (DVE)
                fx0, fxsl = targets[0]
                fst = stage_by_x0[fx0]
                nc.vector.tensor_copy(
                    out=_ap(fst[:], fxsl * 12,
                            [[ROWE, NCELL], [1, 4], [REC, NX], [4, NC_]]),
                    in_=_ap(ps[:], 0, [[2048, NCELL], [512, 4], [3, NX], [1, NC_]]))
                # remaining targets: cheap fp16 stage->stage copies (DVE/ACT)
                for i, (x0, xsl) in enumerate(targets[1:]):
                    st = stage_by_x0[x0]
                    if i == 0:
                        nc.vector.tensor_copy(
                            out=slot_ap(st, xsl), in_=slot_ap(fst, fxsl))
                    else:
                        nc.scalar.copy(
                            out=slot_ap(st, xsl), in_=slot_ap(fst, fxsl))
                # ship completed stages: stage x0 is complete once x == x0+3
                ship = []
                if x >= 3:
                    ship.append(x - 3)
                if x == NX - 1:
                    ship.extend([NCELL - 3, NCELL - 2, NCELL - 1])
                for x0 in ship:
                    if x0 not in stage_by_x0:
                        continue
                    st = stage_by_x0.pop(x0)
                    eng = nc.sync if (x0 % 2 == 0) else nc.gpsimd
                    eng.dma_start(
                        out=_ap(cy[:], x0 * XSTRIDE * REC,
                                [[ROWE, NCELL], [1, ROWE]]),
                        in_=_ap(st[:], 0, [[ROWE, NCELL], [1, ROWE]]),
                    )

    # ---------------- Phase 2: points ----------------
    with ExitStack() as p2:
        recs = p2.enter_context(tc.tile_pool(name="p2_rec", bufs=2))
        prods = p2.enter_context(tc.tile_pool(name="p2_prod", bufs=2))
        touts = p2.enter_context(tc.tile_pool(name="p2_tout", bufs=2))

        cy_flat = _ap(cy[:], 0, [[REC, NRECTOT], [1, REC]])

        for ch in range(NCHUNK):
            # coords layout: [x | z | y] so (u,w) are adjacent for weights
            c3 = coords.tile([128, 3 * P], F32)
            nc.sync.dma_start(out=c3[:, 0:P], in_=xs[:, ch * P:(ch + 1) * P])
            nc.sync.dma_start(out=c3[:, P:2 * P], in_=zs[:, ch * P:(ch + 1) * P])
            nc.sync.dma_start(out=c3[:, 2 * P:3 * P], in_=ys[:, ch * P:(ch + 1) * P])

            # u = (coord+1)*62.5 ; fl = floor(u) (u >= 0) ; fr = u - fl
            nc.vector.tensor_scalar(c3[:], c3[:], 1.0, INV_D, add, mult)
            ci3 = small.tile([128, 3 * P], I32)
            nc.vector.tensor_copy(out=ci3[:], in_=c3[:])
            cf3 = small.tile([128, 3 * P], F32)
            nc.vector.tensor_copy(out=cf3[:], in_=ci3[:])
            fl3 = small.tile([128, 3 * P], F32)
            nc.vector.tensor_tensor(fl3[:], cf3[:], c3[:], mybir.AluOpType.is_gt)
            nc.vector.tensor_tensor(fl3[:], cf3[:], fl3[:], sub)
            fr3 = small.tile([128, 3 * P], F32)
            nc.vector.tensor_tensor(fr3[:], c3[:], fl3[:], sub)
            nc.vector.tensor_scalar(fl3[:], fl3[:], float(NCELL - 1), None, amin)
            frh = small.tile([128, 3 * P], F16)
            nc.vector.tensor_copy(out=frh[:], in_=fr3[:])

            # record index: ix*16000 + iy*128 + iz
            idxf = small.tile([128, P], F32)
            nc.vector.scalar_tensor_tensor(
                idxf[:], fl3[:, 2 * P:3 * P], float(NX), fl3[:, P:2 * P], mult, add)
            nc.vector.scalar_tensor_tensor(
                idxf[:], fl3[:, 0:P], float(XSTRIDE), idxf[:], mult, add)
            idxi = idxp.tile([128, P], I32)
            nc.vector.tensor_copy(out=idxi[:], in_=idxf[:])

            # ---- tap weights for u (x) and w (z): wt [128 | g2, P, k4] fp16 ----
            wt = small.tile([128, 2 * P * 4], F16)
            uw = _ap(frh[:], 0, [[3 * P, 128], [1, 2 * P]])

            def wslice(k):
                return _ap(wt[:], k, [[8 * P, 128], [4 * P, 2], [4, P]])

            tg = small.tile([128, 2 * P], F16)
            t2g = small.tile([128, 2 * P], F16)
            r2 = small.tile([128, 2 * P], F16)
            r3 = small.tile([128, 2 * P], F16)
            tmp = small.tile([128, 2 * P], F16)

            def v2(t):  # view [128, 2P] as (2, P)
                return _ap(t[:], 0, [[2 * P, 128], [P, 2], [1, P]])

            nc.vector.tensor_scalar(tg[:], uw, -1.0, 1.0, mult, add)
            nc.vector.tensor_tensor(t2g[:], tg[:], tg[:], mult)
            nc.vector.scalar_tensor_tensor(wslice(0), v2(t2g), 1 / 6, v2(tg), mult, mult)
            nc.vector.tensor_tensor(r2[:], uw, uw, mult)
            nc.vector.tensor_tensor(r3[:], r2[:], uw, mult)
            nc.vector.tensor_scalar(wslice(3), v2(r3), 1 / 6, None, mult)
            nc.vector.scalar_tensor_tensor(tmp[:], r3[:], 0.5, r2[:], mult, sub)
            nc.vector.tensor_scalar(wslice(1), v2(tmp), 2 / 3, None, add)
            nc.vector.tensor_tensor(v2(tmp), wslice(0), wslice(1), add)
            nc.vector.tensor_tensor(v2(tmp), v2(tmp), wslice(3), add)
            nc.vector.tensor_scalar(wslice(2), v2(tmp), -1.0, 1.0, mult, add)

            # ---- v powers: vp4 [128, P, 4] = [1, v, v^2, v^3] fp16 ----
            vp4 = small.tile([128, P * 4], F16)
            frv = _ap(frh[:], 2 * P, [[3 * P, 128], [1, P]])

            def vslot(k):
                return _ap(vp4[:], k, [[4 * P, 128], [4, P]])

            nc.vector.memset(vslot(0), 1.0)
            nc.vector.tensor_copy(out=vslot(1), in_=frv)
            nc.vector.tensor_tensor(vslot(2), frv, frv, mult)
            nc.vector.tensor_tensor(vslot(3), vslot(2), frv, mult)

            # ---- x weights expanded over c: wuc [128, P, x4, c3] fp16 ----
            wuc = small.tile([128, P * 12], F16)
            nc.vector.tensor_copy(
                out=_ap(wuc[:], 0, [[12 * P, 128], [12, P], [3, 4], [1, 3]]),
                in_=_ap(wt[:], 0, [[8 * P, 128], [4, P], [1, 4], [0, 3]]))

            # ---- gather: one record (z-window, 192 fp16) per point ----
            rec = recs.tile([128, P * 192], F16)
            for t in range(P):
                nc.gpsimd.indirect_dma_start(
                    out=_ap(rec[:], t * 192, [[192 * P, 128], [1, 192]]),
                    out_offset=None,
                    in_=cy_flat,
                    in_offset=bass.IndirectOffsetOnAxis(
                        ap=_ap(idxi[:], t, [[P, 128], [1, 1]]), axis=0),
                )

            # ---- contraction ----
            # per point rec = [z4][x4][c3][k4]
            # k poly-eval: rec[pt, zxc48, k4] *= vp4[pt, k4]; tree-add over k
            nc.vector.tensor_tensor(
                _ap(rec[:], 0, [[192 * P, 128], [192, P], [4, 48], [1, 4]]),
                _ap(rec[:], 0, [[192 * P, 128], [192, P], [4, 48], [1, 4]]),
                _ap(vp4[:], 0, [[4 * P, 128], [4, P], [0, 48], [1, 4]]),
                mult)
            nc.vector.tensor_tensor(
                _ap(rec[:], 0, [[192 * P, 128], [192, P], [4, 48], [1, 2]]),
                _ap(rec[:], 0, [[192 * P, 128], [192, P], [4, 48], [1, 2]]),
                _ap(rec[:], 2, [[192 * P, 128], [192, P], [4, 48], [1, 2]]),
                add)
            s1 = prods.tile([128, P * 48], F16)
            nc.vector.tensor_tensor(
                _ap(s1[:], 0, [[48 * P, 128], [48, P], [1, 48]]),
                _ap(rec[:], 0, [[192 * P, 128], [192, P], [4, 48]]),
                _ap(rec[:], 1, [[192 * P, 128], [192, P], [4, 48]]),
                add)
            # x contraction: s1[pt, z4, (x4 c3)12] *= wuc; tree-add over x
            nc.vector.tensor_tensor(
                _ap(s1[:], 0, [[48 * P, 128], [48, P], [12, 4], [1, 12]]),
                _ap(s1[:], 0, [[48 * P, 128], [48, P], [12, 4], [1, 12]]),
                _ap(wuc[:], 0, [[12 * P, 128], [12, P], [0, 4], [1, 12]]),
                mult)
            nc.vector.tensor_tensor(
                _ap(s1[:], 0, [[48 * P, 128], [48, P], [12, 4], [1, 6]]),
                _ap(s1[:], 0, [[48 * P, 128], [48, P], [12, 4], [1, 6]]),
                _ap(s1[:], 6, [[48 * P, 128], [48, P], [12, 4], [1, 6]]),
                add)
            s2 = prods.tile([128, P * 12], F16)
            nc.vector.tensor_tensor(
                _ap(s2[:], 0, [[12 * P, 128], [12, P], [3, 4], [1, 3]]),
                _ap(s1[:], 0, [[48 * P, 128], [48, P], [12, 4], [1, 3]]),
                _ap(s1[:], 3, [[48 * P, 128], [48, P], [12, 4], [1, 3]]),
                add)
            # z contraction: s2[pt, z4, c3] *= ww (bcast over c); tree-add over z
            nc.vector.tensor_tensor(
                _ap(s2[:], 0, [[12 * P, 128], [12, P], [3, 4], [1, 3]]),
                _ap(s2[:], 0, [[12 * P, 128], [12, P], [3, 4], [1, 3]]),
                _ap(wt[:], 4 * P, [[8 * P, 128], [4, P], [1, 4], [0, 3]]),
                mult)
            nc.vector.tensor_tensor(
                _ap(s2[:], 0, [[12 * P, 128], [12, P], [1, 6]]),
                _ap(s2[:], 0, [[12 * P, 128], [12, P], [1, 6]]),
                _ap(s2[:], 6, [[12 * P, 128], [12, P], [1, 6]]),
                add)
            t_c = touts.tile([128, P * 3], F32)
            nc.vector.tensor_tensor(
                _ap(t_c[:], 0, [[3 * P, 128], [3, P], [1, 3]]),
                _ap(s2[:], 0, [[12 * P, 128], [12, P], [1, 3]]),
                _ap(s2[:], 3, [[12 * P, 128], [12, P], [1, 3]]),
                add)

            nc.sync.dma_start(
                out=t_out[:, ch * P:(ch + 1) * P, :],
                in_=t_c[:].rearrange("p (a b) -> p a b", b=3))


# ======================================================================
# Self-contained entry point: kernel(**inputs) -> np.ndarray
# ======================================================================

N_POINTS = 2_000_000
N_CORES = 8
PTS_PER_CORE = N_POINTS // N_CORES      # 250000
PAD_PER_CORE = 128 * COLS               # 262144

_CACHE = {}


def _build_nc():
    import concourse.bacc as bacc

    nc = bacc.Bacc(
        "TRN2",
        target_bir_lowering=False,
        debug=False,
        num_devices=N_CORES,
    )
    xs = nc.dram_tensor("xs", [128, COLS], F32, kind="ExternalInput").ap()
    ys = nc.dram_tensor("ys", [128, COLS], F32, kind="ExternalInput").ap()
    zs = nc.dram_tensor("zs", [128, COLS], F32, kind="ExternalInput").ap()
    phi = nc.dram_tensor("phi", [128, 128, ZC], F32, kind="ExternalInput").ap()
    t_out = nc.dram_tensor("t_out", [128, COLS, NC_], F32, kind="ExternalOutput").ap()

    with tile.TileContext(nc) as tc:
        bspline_kernel(tc, [t_out], [xs, ys, zs, phi])
    nc.compile()
    return nc


def get_nc():
    if "nc" not in _CACHE:
        _CACHE["nc"] = _build_nc()
    return _CACHE["nc"]


def _shard(arr):
    """[N_POINTS] -> list of 8 [128, COLS] arrays (padded with zeros)."""
    out = []
    for c in range(N_CORES):
        s = arr[c * PTS_PER_CORE:(c + 1) * PTS_PER_CORE]
        p = np.zeros(PAD_PER_CORE, dtype=np.float32)
        p[:PTS_PER_CORE] = s
        out.append(p.reshape(128, COLS))
    return out


def run_on_cores(x, y, z, phi_x, trace=False, **kw):
    from concourse.bass_utils import run_bass_kernel_spmd

    nc = get_nc()
    xsh, ysh, zsh = _shard(x), _shard(y), _shard(z)
    phi_r = np.ascontiguousarray(phi_x.reshape(128, 128, ZC))
    in_maps = [
        {"xs": xsh[c], "ys": ysh[c], "zs": zsh[c], "phi": phi_r}
        for c in range(N_CORES)
    ]
    res = run_bass_kernel_spmd(
        nc, in_maps, core_ids=list(range(N_CORES)), trace=trace, **kw
    )
    outs = []
    for c in range(N_CORES):
        t = res.results[c]["t_out"].reshape(PAD_PER_CORE, NC_)
        outs.append(t[:PTS_PER_CORE])
    full = np.concatenate(outs, axis=0).astype(np.float32)
    return full, res


def kernel(x, y, z, phi_x):
    full, _ = run_on_cores(
        np.asarray(x, dtype=np.float32),
        np.asarray(y, dtype=np.float32),
        np.asarray(z, dtype=np.float32),
        np.asarray(phi_x, dtype=np.float32),
    )
    return full
